# revision 12
# baseline (speedup 1.0000x reference)
"""Multi-head attention (B=4, S=1024, D=1024, H=16) on 8 TRN2 NeuronCores.

Sharding: batch (4) x head-half (2) -> 8 cores, zero cross-core traffic.
Core c handles batch b = c // 2 and heads [hh*8, hh*8+8) where hh = c % 2.
Each core computes a partial output y_part[s, e] (its 512 channels fed
through its slice of Wo) in bf16; the host sums the two partials per batch
and adds the bias terms in f32.

On-device pipeline per core (matmul operands bf16, accumulation fp32):

Head (overlapped with input DMA, d-outer so compute starts on chunk 0):
  V' = xv @ Wv'            [s, 512] natural layout + per-head ones column
  QT0 = Wq_0' @ xq         [128 dout, 1024 s] (weights pre-scaled 1/sqrt(dk))
  KT0 = Wk_0' @ xk

Cruise (8 blocks = 4 head-pairs x 2 q-chunks, software-pipelined):
  block b: scores+exp of b interleaved (in PE program order) with the AV
  matmuls of block b-1 and the Q/K projection d-steps of pair j+1 — the
  exp (ACT) of block b is the pacing op for the score PSUM rotation, so
  the interleaved fillers keep the PE busy during the exp-paced stretch.
    ST   = KhT.T @ QhT           [k 128, q 512]  (K=64, both heads share
                                  one 2-bank PSUM tile -> one exp covers both)
    E    = exp(ST + mask_bias)   (ACT, fused mask)
    psO += Vaug.T @ E            [65, q 512] rows 0-63 = out_h^T, row 64 = denom
  normalize: 1/denom via single-op DVE reciprocal_approx_fast; the
  partition-broadcast and the multiply run on gpsimd (idle engine) so the
  DVE queue never round-trips through gpsimd.
  Block 7's fillers are the first half of the output projection (rows 0-511).

Tail: y = concatT.T @ Wo'  [1024 s, 1024 e], stored as bf16 across 3 queues.
"""

import os
import sys

sys.path.insert(0, "/opt/trn_rl_repo")

import numpy as np
import ml_dtypes

BF16 = ml_dtypes.bfloat16

B, S, D = 4, 1024, 1024
HEADS = 16
DK = 64
P = 128
NCORES = 8
DCH = D // P       # 8 contraction chunks
PAIRS = 4          # head-pairs per core (8 heads / 2)
QN = 2             # q 512-chunks
KT = 8             # k tiles of 128
VW = 65            # V channels per head + ones column

_STATE = {}


def _build():
    """Build + compile the per-core Bass program (cached)."""
    if "nc" in _STATE:
        return _STATE["nc"]

    import concourse.bass as bass  # noqa: F401
    import concourse.mybir as mybir
    from concourse import bacc
    from concourse import tile

    f32 = mybir.dt.float32
    bf16 = mybir.dt.bfloat16
    AF = mybir.ActivationFunctionType
    ALU = mybir.AluOpType

    # Pin Exp/Ln to the one activation table containing both, so the
    # table-load pass never alternates tables between the softmax exp and
    # the ln/exp reciprocal (each ACT_TABLE_LOAD costs ~1.3us). Only the
    # chooser's view is filtered; table ids keep act_info.json order.
    _orig_tables = bacc.get_activation_tables

    def _pinned_tables(arch):
        t = dict(_orig_tables(arch))
        target = "natural_log_exp_and_others"
        if target in t:
            for k in t:
                if k != target:
                    t[k] = t[k] - {AF.Exp, AF.Ln}
        return t

    bacc.get_activation_tables = _pinned_tables

    nc = bacc.Bacc("TRN2", target_bir_lowering=False, debug=False)

    xq_d = nc.dram_tensor("xq", [D, S], bf16, kind="ExternalInput")
    xk_d = nc.dram_tensor("xk", [D, S], bf16, kind="ExternalInput")
    xv_d = nc.dram_tensor("xv", [D, S], bf16, kind="ExternalInput")
    wq_d = nc.dram_tensor("wq", [PAIRS, D, P], bf16, kind="ExternalInput")
    wk_d = nc.dram_tensor("wk", [PAIRS, D, P], bf16, kind="ExternalInput")
    wv_d = nc.dram_tensor("wv", [D, 512], bf16, kind="ExternalInput")
    wo_d = nc.dram_tensor("wo", [512, D], bf16, kind="ExternalInput")
    bq_d = nc.dram_tensor("bq", [P, PAIRS], f32, kind="ExternalInput")
    bk_d = nc.dram_tensor("bk", [P, PAIRS], f32, kind="ExternalInput")
    mb_d = nc.dram_tensor("mb", [P, KT], f32, kind="ExternalInput")
    y_d = nc.dram_tensor("y", [S, D], bf16, kind="ExternalOutput")

    from contextlib import ExitStack

    with tile.TileContext(nc) as tc, ExitStack() as ctx:
        const = ctx.enter_context(tc.tile_pool(name="const", bufs=1))
        # Resident tensors
        wv_sb = const.tile([P, DCH, 512], bf16)
        wq_sb = const.tile([P, PAIRS, DCH, P], bf16)
        wk_sb = const.tile([P, PAIRS, DCH, P], bf16)
        wo_sb = const.tile([P, PAIRS, D], bf16)
        xq_sb = const.tile([P, DCH, S], bf16)
        xk_sb = const.tile([P, DCH, S], bf16)
        xv_sb = const.tile([P, DCH, S], bf16)
        v_sb = const.tile([P, KT, 8 * VW], bf16)
        cat_sb = const.tile([P, PAIRS, S], bf16)
        bq_sb = const.tile([P, PAIRS], f32)
        bk_sb = const.tile([P, PAIRS], f32)
        mb_sb = const.tile([P, KT], f32)

        # SBUF pools
        qtp = ctx.enter_context(tc.tile_pool(name="qtp", bufs=2))
        ktp = ctx.enter_context(tc.tile_pool(name="ktp", bufs=2))
        epool = ctx.enter_context(tc.tile_pool(name="epool", bufs=16))
        rpool = ctx.enter_context(tc.tile_pool(name="rpool", bufs=4))
        r2pool = ctx.enter_context(tc.tile_pool(name="r2pool", bufs=4))
        ypool = ctx.enter_context(tc.tile_pool(name="ypool", bufs=3))
        spool = ctx.enter_context(tc.tile_pool(name="spool", bufs=6))
        # PSUM: 8 banks total. pssp 2x[128,2,512] = 4 banks (scores; also the
        # V accumulators in the head), psop 2x[128,512] = 2 banks (AV output;
        # also Q-p0 accumulators), psacc 2x[128,512] = 2 banks (projections).
        pssp = ctx.enter_context(tc.tile_pool(name="pssp", bufs=2, space="PSUM"))
        psop = ctx.enter_context(tc.tile_pool(name="psop", bufs=2, space="PSUM"))
        psacc = ctx.enter_context(tc.tile_pool(name="psacc", bufs=2, space="PSUM"))

        # ---------------- DMA issuance (priority interleaved) ----------------
        # scalar queue: only small/early transfers so ACT stays clear for exp.
        wv_r = wv_d.ap().rearrange("(d p) m -> d p m", p=P)
        xv_r = xv_d.ap().rearrange("(d p) s -> d p s", p=P)
        xq_r = xq_d.ap().rearrange("(d p) s -> d p s", p=P)
        xk_r = xk_d.ap().rearrange("(d p) s -> d p s", p=P)
        wq_r = wq_d.ap().rearrange("j (d p) m -> j p d m", p=P)
        wk_r = wk_d.ap().rearrange("j (d p) m -> j p d m", p=P)
        for d in range(DCH):
            nc.scalar.dma_start(wv_sb[:, d], wv_r[d])
            nc.sync.dma_start(xv_sb[:, d], xv_r[d])
            nc.gpsimd.dma_start(xq_sb[:, d], xq_r[d])
            nc.gpsimd.dma_start(xk_sb[:, d], xk_r[d])
        for j in range(PAIRS):
            nc.scalar.dma_start(wq_sb[:, j], wq_r[j])
            nc.sync.dma_start(wk_sb[:, j], wk_r[j])
        nc.scalar.dma_start(bq_sb[:], bq_d.ap())
        nc.sync.dma_start(bk_sb[:], bk_d.ap())
        nc.gpsimd.dma_start(mb_sb[:], mb_d.ap())

        # ---------------- Head: V' projection, d-outer ----------------
        # Group A: st 0-3 in two 2-bank pssp tiles; starts on xv chunk 0.
        psva = [pssp.tile([P, 2, 512], f32, tag="s", name=f"psva{t}") for t in range(2)]
        for d in range(DCH):
            for st in range(4):
                nc.tensor.matmul(
                    psva[st // 2][:, st % 2],
                    xv_sb[:, d, st * P : (st + 1) * P],
                    wv_sb[:, d],
                    start=(d == 0),
                    stop=(d == DCH - 1),
                )
        for st in range(4):
            vview = v_sb[:, st].rearrange("p (h c) -> p h c", c=VW)
            nc.vector.tensor_copy(
                vview[:, :, 0:64],
                psva[st // 2][:, st % 2].rearrange("p (h c) -> p h c", c=64),
            )

        # Group B (st 4-7) + Q-p0, d-outer (xv resident by now; paced by xq).
        psvb = [pssp.tile([P, 2, 512], f32, tag="s", name=f"psvb{t}") for t in range(2)]
        psq0 = [psop.tile([P, 512], f32, tag="o", name=f"psq0_{n}") for n in range(QN)]
        for d in range(DCH):
            for st in range(4, 8):
                nc.tensor.matmul(
                    psvb[(st - 4) // 2][:, st % 2],
                    xv_sb[:, d, st * P : (st + 1) * P],
                    wv_sb[:, d],
                    start=(d == 0),
                    stop=(d == DCH - 1),
                )
            for n in range(QN):
                nc.tensor.matmul(
                    psq0[n],
                    wq_sb[:, 0, d],
                    xq_sb[:, d, n * 512 : (n + 1) * 512],
                    start=(d == 0),
                    stop=(d == DCH - 1),
                )
        for st in range(4, 8):
            vview = v_sb[:, st].rearrange("p (h c) -> p h c", c=VW)
            nc.vector.tensor_copy(
                vview[:, :, 0:64],
                psvb[(st - 4) // 2][:, st % 2].rearrange("p (h c) -> p h c", c=64),
            )
        qts = [qtp.tile([P, S], bf16, tag="qt", name=f"qt{j}") for j in range(PAIRS)]
        kts = [ktp.tile([P, S], bf16, tag="kt", name=f"kt{j}") for j in range(PAIRS)]
        for n in range(QN):
            nc.vector.tensor_scalar_add(
                qts[0][:, n * 512 : (n + 1) * 512], psq0[n], bq_sb[:, 0:1]
            )
        # ones columns for the denominator rows (bf16 memset can't stride here)
        ones_f32 = const.tile([P, KT, 8], f32)
        nc.vector.memset(ones_f32[:], 1.0)
        ones_view = v_sb.rearrange("p t (h c) -> p t h c", c=VW)[:, :, :, 64:65]
        nc.vector.tensor_copy(ones_view, ones_f32[:].unsqueeze(3))

        # K-p0, d-outer (paced by xk).
        psk0 = [psacc.tile([P, 512], f32, tag="acc", name=f"psk0_{n}") for n in range(QN)]
        for d in range(DCH):
            for n in range(QN):
                nc.tensor.matmul(
                    psk0[n],
                    wk_sb[:, 0, d],
                    xk_sb[:, d, n * 512 : (n + 1) * 512],
                    start=(d == 0),
                    stop=(d == DCH - 1),
                )
        for n in range(QN):
            nc.vector.tensor_scalar_add(
                kts[0][:, n * 512 : (n + 1) * 512], psk0[n], bk_sb[:, 0:1]
            )

        # wo is needed only by the output projection; issue after the
        # priority streams so it never competes with xq/xk for HBM.
        nc.sync.dma_start(wo_sb[:], wo_d.ap().rearrange("(c p) e -> p c e", p=P))

        # ---------------- Cruise: software-pipelined blocks ----------------
        y_r = y_d.ap().rearrange("(st p) e -> st p e", p=P)
        yq = [nc.sync, nc.gpsimd]

        def proj_gen(j, wsb, bsb, out_tile, bias_col):
            """Yield one projection matmul per next(); evict after each group."""
            for n in range(QN):
                ps = psacc.tile([P, 512], f32, tag="acc", name=f"pp{j}_{bias_col}_{n}")
                for d in range(DCH):
                    yield nc.tensor.matmul(
                        ps,
                        wsb[:, j, d],
                        xq_sb[:, d, n * 512 : (n + 1) * 512]
                        if wsb is wq_sb
                        else xk_sb[:, d, n * 512 : (n + 1) * 512],
                        start=(d == 0),
                        stop=(d == DCH - 1),
                    )
                nc.vector.tensor_scalar_add(
                    out_tile[:, n * 512 : (n + 1) * 512], ps, bsb[:, j : j + 1]
                )

        def av_gen(b, ets, psos):
            """Yield the 16 AV matmuls of block b (sub-interleaved per kt)."""
            j = b // 2
            for kt in range(KT):
                for sub in range(2):
                    h = j * 2 + sub
                    yield nc.tensor.matmul(
                        psos[sub],
                        v_sb[:, kt, h * VW : (h + 1) * VW],
                        ets[kt][:, sub],
                        start=(kt == 0),
                        stop=(kt == KT - 1),
                    )

        def o_gen():
            """Yield the output-projection matmuls, rows 0-511 first."""
            for st in range(KT):
                for en in range(2):
                    psy = psacc.tile([P, 512], f32, tag="acc", name=f"psy{st}_{en}")
                    for cc in range(PAIRS):
                        yield nc.tensor.matmul(
                            psy,
                            cat_sb[:, cc, st * P : (st + 1) * P],
                            wo_sb[:, cc, en * 512 : (en + 1) * 512],
                            start=(cc == 0),
                            stop=(cc == PAIRS - 1),
                        )
                    ysb = ypool.tile([P, 512], bf16, tag="y", name=f"y{st}_{en}")
                    nc.vector.tensor_copy(ysb[:], psy)
                    yq[(st * 2 + en) % len(yq)].dma_start(
                        y_r[st][:, en * 512 : (en + 1) * 512], ysb[:]
                    )

        def norm(b, psos):
            """Normalize block b: evict psO, 1/denom as exp(-ln(denom)) on ACT
            (one shared Exp covers both subs), partition-broadcast on gpsimd
            (its only Q7 op type, so the library stays resident — mixing Q7 op
            types swaps libs at ~7us each), multiply on DVE."""
            j, qn = b // 2, b % 2
            stgs, r2s = [], []
            for sub in range(2):
                stg = spool.tile([VW, 512], f32, tag="stg", name=f"stg{b}_{sub}")
                nc.vector.tensor_copy(stg[:], psos[sub][:])
                stgs.append(stg)
            for sub in range(2):
                lrow = rpool.tile([1, 512], f32, tag="l", name=f"l{b}_{sub}")
                nc.scalar.activation(lrow[:], psos[sub][64:65, :], AF.Ln)
                rrow = rpool.tile([1, 512], f32, tag="r", name=f"r{b}_{sub}")
                nc.scalar.activation(rrow[:], lrow[:], AF.Exp, scale=-1.0)
                r2 = r2pool.tile([64, 512], f32, tag="r2", name=f"r2{b}_{sub}")
                nc.gpsimd.partition_broadcast(r2[:], rrow[:])
                r2s.append(r2)
            for sub in range(2):
                lo = sub * 64
                nc.vector.tensor_tensor(
                    cat_sb[lo : lo + 64, j, qn * 512 : (qn + 1) * 512],
                    stgs[sub][0:64, :],
                    r2s[sub][:],
                    op=ALU.mult,
                )

        o_filler = o_gen()
        prev = None  # (block index, ets, psos) of the block awaiting AV
        for b in range(8):
            j, qn = b // 2, b % 2
            fillers = []
            per_round = 2
            if b == 7:
                # Last block: drain AV+normalize of block 6 up front so the
                # O-projection (rows 0-511, which need every pair's qn0 cat)
                # can serve as this block's score-stretch filler.
                psos = [
                    psop.tile([VW, 512], f32, tag="o", name=f"pso{prev[0]}_{s}")
                    for s in range(2)
                ]
                for _ in av_gen(prev[0], prev[1], psos):
                    pass
                norm(prev[0], psos)
                prev = None
                fillers.append(o_filler)
                per_round = 4
            else:
                if prev is not None:
                    psos = [
                        psop.tile([VW, 512], f32, tag="o", name=f"pso{prev[0]}_{s}")
                        for s in range(2)
                    ]
                    fillers.append(av_gen(prev[0], prev[1], psos))
                    prev = (prev[0], prev[1], psos)
                if j < 3:
                    nj = j + 1
                    if qn == 0:
                        fillers.append(proj_gen(nj, wq_sb, bq_sb, qts[nj], nj))
                    else:
                        fillers.append(proj_gen(nj, wk_sb, bk_sb, kts[nj], nj))

            ets = []
            for kt in range(KT):
                pss = pssp.tile([P, 2, 512], f32, tag="s", name=f"pss{b}_{kt}")
                for sub in range(2):
                    lo, hi = sub * 64, (sub + 1) * 64
                    nc.tensor.matmul(
                        pss[:, sub],
                        kts[j][lo:hi, kt * P : (kt + 1) * P],
                        qts[j][lo:hi, qn * 512 : (qn + 1) * 512],
                        start=True,
                        stop=True,
                    )
                et = epool.tile([P, 2, 512], bf16, tag="e", name=f"e{b}_{kt}")
                nc.scalar.activation(
                    et[:], pss[:], AF.Exp, bias=mb_sb[:, kt : kt + 1], scale=1.0
                )
                ets.append(et)
                # interleave filler matmuls per kt round so the PE stays
                # busy while the exp-paced score-PSUM rotation catches up
                for f in fillers:
                    for _ in range(per_round):
                        next(f, None)
            if prev is not None:
                for f in fillers[:1]:
                    for _ in iter(lambda: next(f, None), None):
                        pass  # drain any remaining AV matmuls of prev block
                norm(prev[0], prev[2])
            if j < 3:
                for _ in iter(lambda: next(fillers[-1], None), None):
                    pass  # drain remaining projection matmuls
            prev = (b, ets, None)

        # Drain: AV + normalize of block 7, then the qn1-dependent O rows.
        psos = [psop.tile([VW, 512], f32, tag="o", name=f"pso7_{s}") for s in range(2)]
        for _ in av_gen(7, prev[1], psos):
            pass
        norm(7, psos)
        for _ in o_filler:
            pass

    nc.compile()
    _STATE["nc"] = nc
    return nc


def _shard(q, k, v, mask, Wq, bq, Wk, bk, Wv, bv, Wo, bo):
    """Build the 8 per-core input maps (host-side layout preparation)."""
    scale = 1.0 / np.sqrt(DK)
    in_maps = []
    for c in range(NCORES):
        b = c // 2
        hh = c % 2
        c0 = hh * 512
        wq_s = (Wq[c0 : c0 + 512, :] * scale).T  # [D, 512]
        wk_s = Wk[c0 : c0 + 512, :].T
        wv_s = Wv[c0 : c0 + 512, :].T
        wo_s = Wo[:, c0 : c0 + 512].T  # [512, D]
        mrow = mask[b, 0, 0, :]
        in_maps.append(
            {
                "xq": np.ascontiguousarray(q[b].T).astype(BF16),
                "xk": np.ascontiguousarray(k[b].T).astype(BF16),
                "xv": np.ascontiguousarray(v[b].T).astype(BF16),
                "wq": np.ascontiguousarray(
                    wq_s.reshape(D, PAIRS, P).transpose(1, 0, 2)
                ).astype(BF16),
                "wk": np.ascontiguousarray(
                    wk_s.reshape(D, PAIRS, P).transpose(1, 0, 2)
                ).astype(BF16),
                "wv": np.ascontiguousarray(wv_s).astype(BF16),
                "wo": np.ascontiguousarray(wo_s).astype(BF16),
                "bq": np.ascontiguousarray(
                    (bq[c0 : c0 + 512] * scale).reshape(PAIRS, P).T, dtype=np.float32
                ),
                "bk": np.ascontiguousarray(
                    bk[c0 : c0 + 512].reshape(PAIRS, P).T, dtype=np.float32
                ),
                "mb": np.ascontiguousarray(
                    np.where(mrow == 0, np.float32(-1e9), np.float32(0.0))
                    .astype(np.float32)
                    .reshape(KT, P)
                    .T
                ),
            }
        )
    return in_maps


def _gather(results, Wv, bv, Wo, bo):
    """Sum per-core bf16 partials into the full [B, S, D] f32 output."""
    # Channel-bias correction folded out of the device kernel: the V bias
    # passes through softmax-weighted sums with total weight 1, so its
    # contribution to y is the constant row Wo @ bv.
    corr = (Wo.astype(np.float64) @ bv.astype(np.float64)).astype(np.float32)
    y = np.empty((B, S, D), dtype=np.float32)
    for b in range(B):
        y[b] = (
            results[2 * b]["y"].astype(np.float32)
            + results[2 * b + 1]["y"].astype(np.float32)
            + corr
            + bo
        )
    return y


def _run(trace=False, **inputs):
    import time

    from concourse.bass_utils import run_bass_kernel_spmd

    nc = _build()
    args = {k: np.asarray(v) for k, v in inputs.items()}
    in_maps = _shard(**args)
    last_err = None
    for attempt in range(3):
        try:
            res = run_bass_kernel_spmd(
                nc, in_maps, core_ids=list(range(NCORES)), trace=trace
            )
            break
        except Exception as e:  # device occasionally wedges; retry recovers
            last_err = e
            time.sleep(10 * (attempt + 1))
    else:
        raise last_err
    y = _gather(res.results, args["Wv"], args["bv"], args["Wo"], args["bo"])
    return y, res


def kernel(**inputs):
    y, _ = _run(trace=False, **inputs)
    return y
